# revision 14
# baseline (speedup 1.0000x reference)
"""FAGCN (2-layer, eval mode) on 8 Trainium2 NeuronCores.

Self-contained kernel: takes FULL inputs, shards across 8 cores internally,
runs one SPMD Bass program, gathers the full output.

Math (per layer, derived from the reference):
  norm_e = dinv[row_e] * dinv[col_e]  factorizes, so with hs_i = dinv_i * h_i:
    x_l[c] = dinv_c * ( sum_{e->c} hs[row_e] + hs_c )        (self-loop folded in)
    gate_i = sigmoid(h_i . att_l)
    h_new  = (gate*(1+eps) - eps) * x_l                       (since x_h = -eps*x_l)

Device strategy per core (6250 owned nodes):
  - MLP: h0 = relu(x W1 + b1) on owned nodes (x passed transposed, bf16).
  - hs tables (bf16, 128-wide zero-padded rows = 256B for the gather elem
    constraint) exchanged with two AllGathers per layer, triggered from the
    scalar engine so they never queue behind gathers.
  - Propagate: dma_gather 256B rows per edge, round-robined over all 4 SWDGE
    queues (each queue runs on its own Q7 core pair, so descriptor generation
    overlaps 4-way) -> PE one-hot segment-sum (bf16) into PSUM windows of 128
    target nodes; flush adds self-loop rows; epilogue applies gate scales.
  - Output: h2 tiles PE-transposed, out^T = W2^T @ h2^T + b2, host transposes.
"""

import math
import os
import sys

import numpy as np

for _p in ("/opt/trn_rl_repo",):
    if _p not in sys.path:
        sys.path.insert(0, _p)

import ml_dtypes

BF16 = ml_dtypes.bfloat16

# ----------------------------------------------------------------------------
# Problem constants (hardcoded per the harness contract)
# ----------------------------------------------------------------------------
N_NODES = 50000
N_EDGES = 800000
EPS = 0.1
NC = 8              # cores
F_IN = 256          # input features
H = 64              # hidden
F_OUT = 16
NPC = N_NODES // NC  # 6250 nodes per core
NT = (NPC + 127) // 128          # 49 windows / node tiles per core
NPADR = NT * 128                 # 6272 padded rows per core
WA = 25                          # windows in half A
ROWS_A = WA * 128                # 3200  (8*3200 = 25600 < 32767 for int16 idx)
ROWS_B = NPC - ROWS_A            # 3050  (8*3050 = 24400)
CH_SLICES = 8                    # slices per gather chunk (1024 edges)
NBUF = 8                         # gather bounce buffers (multiple of NQ)
NQ = 4                           # SWDGE queues used round-robin
RING = 32                        # one-hot S-tile ring depth (slices)
CHN = 512                        # MLP node-chunk (columns of x^T per load)
TW = 128                         # table row width (bf16), 256B rows


def set_config(n_nodes, n_edges, ch_slices=None):
    """Recompute derived sizes (used by tests to run small configs)."""
    global N_NODES, N_EDGES, NPC, NT, NPADR, WA, ROWS_A, ROWS_B, CH_SLICES
    ch_slices = ch_slices or CH_SLICES
    N_NODES, N_EDGES = n_nodes, n_edges
    NPC = N_NODES // NC
    NT = (NPC + 127) // 128
    NPADR = NT * 128
    WA = (NT + 1) // 2
    ROWS_A = min(WA * 128, NPC)
    ROWS_B = NPC - ROWS_A
    CH_SLICES = ch_slices
    assert ROWS_B > 0 and NC * ROWS_A < 32768


class _Plan:
    """Static (SPMD-identical) schedule + per-core input arrays."""
    pass


def _preprocess(x, edge_index, W1, b1, att, W2, b2):
    p = _Plan()
    row = np.asarray(edge_index[0], dtype=np.int64)
    col = np.asarray(edge_index[1], dtype=np.int64)

    deg = np.bincount(col, minlength=N_NODES).astype(np.float64) + 1.0
    dinv = (1.0 / np.sqrt(deg)).astype(np.float32)

    owner = col // NPC
    cl = (col % NPC).astype(np.int64)
    w = cl // 128
    so = (row % NPC).astype(np.int64)
    sown = row // NPC
    half = (so >= ROWS_A).astype(np.int64)
    gidx = np.where(half == 0, sown * ROWS_A + so, sown * ROWS_B + (so - ROWS_A))
    assert gidx.max() < 32768

    # sort edges by (owner, half, window)
    order = np.lexsort((w, half, owner))
    so_row, so_gidx, so_cl, so_w = (
        owner[order], gidx[order], cl[order], w[order])
    so_half = half[order]

    # group counts per (core, half, window)
    key = (so_row * 2 + so_half) * NT + so_w
    cnt = np.bincount(key, minlength=NC * 2 * NT).reshape(NC, 2, NT)
    nsl = np.maximum(1, (cnt.max(axis=0) + 127) // 128)   # [2, NT] slices/group
    p.nsl = nsl
    p.group_slices = []   # per group g: (start_slice, n_slices)
    acc = 0
    for hh in range(2):
        for ww in range(NT):
            ns = int(nsl[hh, ww])
            p.group_slices.append((acc, ns))
            acc += ns
    p.TOT_SLICES = acc
    p.SA_SLICES = int(nsl[0].sum())
    assert int(nsl.max()) <= RING // 2 - 2, f"window too big: {nsl.max()}"

    # slice -> group map, slice -> chunk map; chunks never straddle halves
    p.slice_group = np.empty(acc, dtype=np.int64)
    for g, (s0, ns) in enumerate(p.group_slices):
        p.slice_group[s0:s0 + ns] = g
    p.chunks = []          # (half, start_slice, n_slices)
    for hh, (lo, hi) in enumerate([(0, p.SA_SLICES), (p.SA_SLICES, acc)]):
        s = lo
        while s < hi:
            ns = min(CH_SLICES, hi - s)
            p.chunks.append((hh, s, ns))
            s += ns
    p.slice_chunk = np.empty(acc, dtype=np.int64)
    p.chunk_start = np.empty(len(p.chunks), dtype=np.int64)
    for ci, (hh, s0, ns) in enumerate(p.chunks):
        p.slice_chunk[s0:s0 + ns] = ci
        p.chunk_start[ci] = s0

    # per-core padded edge arrays in schedule order
    p.ri = np.zeros((NC, 128, p.TOT_SLICES * 8), dtype=np.int16)
    p.colw = np.full((NC, 128, p.TOT_SLICES), -1.0, dtype=np.float32)
    # per (core, half, window) extract sorted segment boundaries
    seg_starts = np.zeros(NC * 2 * NT + 1, dtype=np.int64)
    np.cumsum(cnt.reshape(-1), out=seg_starts[1:])
    ri_flat = np.zeros((NC, p.TOT_SLICES * 128), dtype=np.int16)
    colw_flat = np.full((NC, p.TOT_SLICES * 128), -1.0, dtype=np.float32)
    for c in range(NC):
        for hh in range(2):
            for ww in range(NT):
                k = (c * 2 + hh) * NT + ww
                a, b = seg_starts[k], seg_starts[k + 1]
                g = hh * NT + ww
                s0, ns = p.group_slices[g]
                n = b - a
                assert n <= ns * 128
                ri_flat[c, s0 * 128: s0 * 128 + n] = so_gidx[a:b]
                colw_flat[c, s0 * 128: s0 * 128 + n] = (
                    so_cl[a:b] - 128 * ww).astype(np.float32)
        # wrap ri per chunk: arr[p, q] = edge[q*16+p] within the chunk
        for ci, (hh, s0, ns) in enumerate(p.chunks):
            blk = ri_flat[c, s0 * 128:(s0 + ns) * 128]
            wrapped = blk.reshape(ns * 8, 16).T          # [16, ns*8]
            p.ri[c, :, s0 * 8:(s0 + ns) * 8] = np.tile(wrapped, (8, 1))
        p.colw[c] = colw_flat[c].reshape(p.TOT_SLICES, 128).T

    # per-core dense inputs
    p.xt = np.zeros((NC, 2, 128, NPADR), dtype=BF16)
    p.dinv = np.zeros((NC, 128, NT), dtype=np.float32)
    x = np.asarray(x, dtype=np.float32)
    for c in range(NC):
        xs = x[c * NPC:(c + 1) * NPC]                    # [6250, 256]
        xts = np.zeros((F_IN, NPADR), dtype=np.float32)
        xts[:, :NPC] = xs.T
        p.xt[c] = xts.reshape(2, 128, NPADR).astype(BF16)
        dv = np.zeros(NPADR, dtype=np.float32)
        dv[:NPC] = dinv[c * NPC:(c + 1) * NPC]
        p.dinv[c] = dv.reshape(NT, 128).T

    # shared weights / constants
    W1 = np.asarray(W1, dtype=np.float32)
    p.w1 = np.concatenate([W1[:128, :], W1[128:, :]], axis=1).astype(BF16)
    p.b1bc = np.tile(np.asarray(b1, dtype=np.float32)[None, :], (128, 1))
    att = np.asarray(att, dtype=np.float32).reshape(2, H)
    p.attbc = np.tile(att.reshape(1, 2 * H), (128, 1))          # [128, 2H]
    p.w2 = np.asarray(W2, dtype=np.float32).astype(BF16)        # [64, 16]
    p.b2c = np.asarray(b2, dtype=np.float32).reshape(F_OUT, 1)
    p.ident = np.eye(128, dtype=np.float32)
    p.iota = np.tile(np.arange(128, dtype=np.float32)[None, :],
                     (128, 1)).astype(BF16)
    return p


# ----------------------------------------------------------------------------
# Bass program
# ----------------------------------------------------------------------------

def build_program(p):
    from concourse import bass, mybir
    from concourse import library_config, library_overlay

    f32 = mybir.dt.float32
    bf16 = mybir.dt.bfloat16
    i16 = mybir.dt.int16
    Alu = mybir.AluOpType
    Act = mybir.ActivationFunctionType

    nc = bass.Bass(trn_type="TRN2", num_devices=NC, num_swdge_queues=NQ)
    cores = list(range(NC))

    # ---- external I/O
    xt_e = nc.declare_dram_parameter("xt", [2, 128, NPADR], bf16, isOutput=False)
    w1_e = nc.declare_dram_parameter("w1", [128, 2 * H], bf16, isOutput=False)
    b1_e = nc.declare_dram_parameter("b1bc", [128, H], f32, isOutput=False)
    att_e = nc.declare_dram_parameter("attbc", [128, 2 * H], f32, isOutput=False)
    w2_e = nc.declare_dram_parameter("w2", [H, F_OUT], bf16, isOutput=False)
    b2_e = nc.declare_dram_parameter("b2c", [F_OUT, 1], f32, isOutput=False)
    id_e = nc.declare_dram_parameter("ident", [128, 128], f32, isOutput=False)
    io_e = nc.declare_dram_parameter("iota", [128, 128], bf16, isOutput=False)
    dv_e = nc.declare_dram_parameter("dinv", [128, NT], f32, isOutput=False)
    ri_e = nc.declare_dram_parameter("ri", [128, p.TOT_SLICES * 8], i16,
                                     isOutput=False)
    cw_e = nc.declare_dram_parameter("colw", [128, p.TOT_SLICES], f32,
                                     isOutput=False)
    out_e = nc.declare_dram_parameter("outT", [F_OUT, NPADR], f32, isOutput=True)

    # ---- internal DRAM (bf16 tables, 256B rows)
    hsA = [nc.dram_tensor(f"hs{L}A", [ROWS_A, TW], bf16) for L in range(2)]
    hsB = [nc.dram_tensor(f"hs{L}B", [ROWS_B, TW], bf16) for L in range(2)]
    tA = [nc.dram_tensor(f"t{L}A", [NC * ROWS_A, TW], bf16, addr_space="Shared")
          for L in range(2)]
    tB = [nc.dram_tensor(f"t{L}B", [NC * ROWS_B, TW], bf16, addr_space="Shared")
          for L in range(2)]

    ctx_tensors = []

    def sb(name, shape, dt=f32):
        t = nc.sbuf_tensor(name, shape, dt)
        ctx_tensors.append(t)
        return t.__enter__()

    def ps(name):
        t = nc.psum_tensor(name, [128, 512], f32)
        ctx_tensors.append(t)
        return t.__enter__()

    def sem(name):
        s = nc.semaphore(name)
        ctx_tensors.append(s)
        return s.__enter__()

    # ---- SBUF
    xc = [sb(f"xc{i}", [128, 2, CHN], bf16) for i in range(2)]
    w1_s = sb("w1s", [128, 2 * H], bf16)
    b1_s = sb("b1s", [128, H])
    att_s = sb("atts", [128, 2 * H])
    w2_s = sb("w2s", [H, F_OUT], bf16)
    b2_s = sb("b2s", [F_OUT, 1])
    id_s = sb("ids", [128, 128])
    io_s = sb("ios", [128, 128], bf16)
    dv_s = sb("dvs", [128, NT])
    ri_s = sb("ris", [128, p.TOT_SLICES * 8], i16)
    cw_s = sb("cws", [128, p.TOT_SLICES])
    gb = [sb(f"gb{i}", [128, CH_SLICES, TW], bf16) for i in range(NBUF)]
    S_s = sb("Ss", [128, RING, 128], bf16)
    SA_s = sb("SAs", [128, NT, H])
    hs_s = sb("hss", [128, NT, TW], bf16)      # cols H..TW stay zero
    h2_s = sb("h2s", [128, NT, H])
    h0sc = sb("h0sc", [128, H])
    ttsc = sb("ttsc", [128, H])
    d_s = sb("ds", [128, NT])
    dt_s = sb("dts", [128, NT])
    g1d_s = sb("g1ds", [128, NT])
    g1d2_s = sb("g1d2s", [128, NT])
    g2d_s = sb("g2ds", [128, NT])
    gsc = sb("gsc", [128, NT])
    h2T_s = sb("h2Ts", [H, NPADR], bf16)
    oT_s = sb("oTs", [F_OUT, NPADR])

    pseg = [ps(f"pg{i}") for i in range(4)]
    ptr = [ps(f"pt{i}") for i in range(2)]
    pout = [ps(f"po{i}") for i in range(2)]

    # ---- semaphores
    s_ld = sem("s_ld")
    s_xtp = [sem("s_xt0"), sem("s_xt1")]
    s_pe1 = sem("s_pe1")
    s_hs = sem("s_hs")
    s_act1 = sem("s_act1")
    s_cc = sem("s_cc")
    s_hsdA = [sem(f"s_hsdA{L}") for L in range(2)]
    s_hsdB = [sem(f"s_hsdB{L}") for L in range(2)]
    s_g = [[sem(f"s_g{L}_{i}") for i in range(NBUF)] for L in range(2)]
    s_oh = [sem(f"s_oh{L}") for L in range(2)]
    s_mm = [sem(f"s_mm{L}") for L in range(2)]
    s_fl = [sem(f"s_fl{L}") for L in range(2)]
    s_act2 = sem("s_act2")
    s_tr = sem("s_tr")
    s_h2c = sem("s_h2c")
    s_om = sem("s_om")
    s_oc = sem("s_oc")
    s_out = sem("s_out")

    NG = 2 * NT          # groups per layer

    def grp_hw(g):
        return (0, g) if g < NT else (1, g - NT)

    # hs DMA out, one 2D DMA per window (A = windows [0, WA), B = the rest,
    # last window partial). Each increments s_hsd[L] by 16; A done at 16*WA,
    # all done at 16*NT.
    BP = ROWS_B - (NT - 1 - WA) * 128      # rows in the partial last window

    def emit_hs_dma(s, L, gate_sem, base=0):
        for ww in range(NT):
            s.wait_ge(gate_sem, base + ww + 1)
            if ww < WA:
                dst = hsA[L][ww * 128:(ww + 1) * 128, :]
                src = hs_s[:, ww, :]
            elif ww < NT - 1:
                r0 = (ww - WA) * 128
                dst = hsB[L][r0:r0 + 128, :]
                src = hs_s[:, ww, :]
            else:
                r0 = (ww - WA) * 128
                dst = hsB[L][r0:r0 + BP, :]
                src = hs_s[0:BP, ww, :]
            s.dma_start(out=dst, in_=src).then_inc(
                s_hsdA[L] if ww < WA else s_hsdB[L], 16)

    # MLP chunk bookkeeping
    mlp_chunks = []
    c0 = 0
    while c0 < NPADR:
        cw = min(CHN, NPADR - c0)
        mlp_chunks.append((c0, cw))
        c0 += cw
    tiles_before_chunk = [0]
    for (c0, cw) in mlp_chunks:
        tiles_before_chunk.append(tiles_before_chunk[-1] + cw // 128)

    # ======================================================================
    # Block 0: constant loads + hs pad-zeroing
    # ======================================================================
    with nc.Block() as block:
        @block.gpsimd
        def _(g):
            g.load_library(library_config.mlp)

        @block.vector
        def _(v):
            # zero the pad columns of hs_s once; epilogues only write [0, H)
            v.memset(hs_s[:, :, H:TW], 0.0)

        @block.sync
        def _(s):
            n = 0
            for dst, src in [
                (w1_s[:], w1_e[:]), (b1_s[:], b1_e[:]), (att_s[:], att_e[:]),
                (w2_s[:], w2_e[:]), (b2_s[:], b2_e[:]), (id_s[:], id_e[:]),
                (io_s[:], io_e[:]), (dv_s[:], dv_e[:]), (ri_s[:], ri_e[:]),
                (cw_s[:], cw_e[:]),
            ]:
                s.dma_start(out=dst, in_=src).then_inc(s_ld, 16)
                n += 16
            s.wait_ge(s_ld, n)

    # ======================================================================
    # Block 1: MLP -> hs0, gate scales, hs0 DMA out, AllGathers for layer 1
    # ======================================================================
    with nc.Block() as block:
        @block.sync
        def _(s):
            for j, (c0, cw) in enumerate(mlp_chunks):
                if j >= 2:
                    s.wait_ge(s_pe1, tiles_before_chunk[j - 1])
                for hh in range(2):
                    s.dma_start(out=xc[j % 2][:, hh, 0:cw],
                                in_=xt_e[hh, :, c0:c0 + cw]
                                ).then_inc(s_xtp[j % 2], 16)
            # hs0 halves out (after DVE produced them)
            emit_hs_dma(s, 0, s_hs)

        @block.tensor
        def _(t):
            for ti in range(NT):
                j = None
                for jj, (c0, cw) in enumerate(mlp_chunks):
                    if ti * 128 >= c0 and ti * 128 < c0 + cw:
                        j = jj
                        u = (ti * 128 - c0) // 128
                        break
                t.wait_ge(s_xtp[j % 2], 32 * (j // 2 + 1))
                if ti >= 4:
                    t.wait_ge(s_hs, ti - 3)
                t.matmul(pseg[ti % 4][:, 0:H],
                         xc[j % 2][:, 0, u * 128:(u + 1) * 128],
                         w1_s[:, 0:H], start=True, stop=False)
                t.matmul(pseg[ti % 4][:, 0:H],
                         xc[j % 2][:, 1, u * 128:(u + 1) * 128],
                         w1_s[:, H:2 * H], start=False, stop=True
                         ).then_inc(s_pe1, 1)

        @block.vector
        def _(v):
            for ti in range(NT):
                v.wait_ge(s_pe1, ti + 1)
                if ti >= 1:
                    v.wait_ge(s_hs, ti)       # self-sync scratch reuse
                v.tensor_add(h0sc[:], pseg[ti % 4][:, 0:H], b1_s[:])
                v.drain()
                v.tensor_scalar_max(h0sc[:], h0sc[:], 0.0)
                v.drain()
                v.tensor_mul(ttsc[:], h0sc[:], att_s[:, 0:H])
                v.drain()
                v.tensor_reduce(d_s[:, ti:ti + 1], ttsc[:],
                                mybir.AxisListType.X, Alu.add)
                v.tensor_scalar_mul(hs_s[:, ti, 0:H], h0sc[:],
                                    dv_s[:, ti:ti + 1]).then_inc(s_hs, 1)
            v.wait_ge(s_act1, 1)
            v.tensor_scalar(gsc[:], gsc[:], 1.0 + EPS, -EPS, Alu.mult, Alu.add)
            v.drain()
            v.tensor_mul(g1d_s[:], gsc[:], dv_s[:])
            v.drain()
            v.tensor_mul(g1d2_s[:], g1d_s[:], dv_s[:])

        @block.scalar
        def _(a):
            a.wait_ge(s_hs, NT)
            a.activation(gsc[:], d_s[:], Act.Sigmoid).then_inc(s_act1, 1)

        @block.gpsimd
        def _(g):
            g.wait_ge(s_hsdA[0], 16 * WA)
            g.collective_compute(
                "AllGather", Alu.bypass, replica_groups=[cores],
                ins=[hsA[0][:]], outs=[tA[0][:]]).then_inc(s_cc, 1)
            g.wait_ge(s_hsdB[0], 16 * (NT - WA))
            g.collective_compute(
                "AllGather", Alu.bypass, replica_groups=[cores],
                ins=[hsB[0][:]], outs=[tB[0][:]]).then_inc(s_cc, 1)

    # ======================================================================
    # Blocks 2/3: edge layers
    # ======================================================================
    def edge_layer(L):
        att_ap = att_s[:, H:2 * H]      # att for NEXT gate (layer L+1)
        with nc.Block() as block:
            @block.gpsimd
            def _(g):
                first_b = True
                size_regs = {}
                for ci, (hh, s0, ns) in enumerate(p.chunks):
                    if ns * 128 not in size_regs:
                        size_regs[ns * 128] = g.to_reg(ns * 128)
                    if ci == 0:
                        g.wait_ge(s_cc, 2 * L + 1)
                    if hh == 1 and first_b:
                        g.wait_ge(s_cc, 2 * L + 2)
                        first_b = False
                    if ci >= NBUF:
                        last_sl = int(p.chunk_start[ci - NBUF]
                                      + p.chunks[ci - NBUF][2] - 1)
                        g.wait_ge(s_mm[L], int(p.slice_group[last_sl]) + 1)
                    tab = (tA, tB)[hh][L]
                    g.dma_gather(
                        gb[ci % NBUF][:, 0:ns, :], tab[:],
                        ri_s[:, s0 * 8:(s0 + ns) * 8],
                        ns * 128, size_regs[ns * 128], TW,
                        queue_num=ci % NQ,
                        ).then_inc(s_g[L][ci % NBUF], 16)
                # issue next layer's AllGathers (after hs(L+1) DMA'd out)
                if L == 0:
                    g.wait_ge(s_hsdA[1], 16 * WA)
                    g.collective_compute(
                        "AllGather", Alu.bypass, replica_groups=[cores],
                        ins=[hsA[1][:]], outs=[tA[1][:]]).then_inc(s_cc, 1)
                    g.wait_ge(s_hsdB[1], 16 * (NT - WA))
                    g.collective_compute(
                        "AllGather", Alu.bypass, replica_groups=[cores],
                        ins=[hsB[1][:]], outs=[tB[1][:]]).then_inc(s_cc, 1)

            @block.tensor
            def _(t):
                last_g_thresh = [[0] for _ in range(NBUF)]
                last_oh = [0]

                def wge(eng, se, th, tracker):
                    if th > tracker[0]:
                        eng.wait_ge(se, th)
                        tracker[0] = th
                for g in range(NG):
                    hh, ww = grp_hw(g)
                    s0, nsg = p.group_slices[g]
                    if g >= 4:
                        t.wait_ge(s_fl[L], g - 3)
                    for i in range(nsg):
                        sl = s0 + i
                        ci = int(p.slice_chunk[sl])
                        wge(t, s_g[L][ci % NBUF], 16 * (ci // NBUF + 1),
                            last_g_thresh[ci % NBUF])
                        wge(t, s_oh[L], sl + 1, last_oh)
                        j = sl - int(p.chunk_start[ci])
                        mm = t.matmul(
                            pseg[g % 4][:, 0:H],
                            S_s[:, sl % RING, :],
                            gb[ci % NBUF][:, j, 0:H],
                            start=(i == 0), stop=(i == nsg - 1))
                        if i == nsg - 1:
                            mm.then_inc(s_mm[L], 1)
                # layer-2 extras: transposes + final matmul
                if L == 1:
                    for ww in range(NT):
                        t.wait_ge(s_fl[1], NT + ww + 1)
                        if ww >= 2:
                            t.wait_ge(s_h2c, ww - 1)
                        t.transpose(ptr[ww % 2][0:H, 0:128], h2_s[:, ww, :],
                                    id_s[:]).then_inc(s_tr, 1)
                    for j, (c0, cw) in enumerate(mlp_chunks):
                        t.wait_ge(s_h2c, min(NT, tiles_before_chunk[j + 1]))
                        if j >= 2:
                            t.wait_ge(s_oc, j - 1)
                        t.matmul(pout[j % 2][0:F_OUT, 0:cw], w2_s[:],
                                 h2T_s[:, c0:c0 + cw], start=True, stop=True
                                 ).then_inc(s_om, 1)

            @block.vector
            def _(v):
                def flush(g):
                    # s_fl[L] increments exactly once per group, on the LAST
                    # DVE instruction touching that group's psum/SA chain.
                    hh, ww = grp_hw(g)
                    v.wait_ge(s_mm[L], g + 1)
                    if hh == 0:
                        v.tensor_add(SA_s[:, ww, :], pseg[g % 4][:, 0:H],
                                     hs_s[:, ww, 0:H]).then_inc(s_fl[L], 1)
                    else:
                        v.tensor_add(SA_s[:, ww, :], SA_s[:, ww, :],
                                     pseg[g % 4][:, 0:H])
                        v.drain()
                        # epilogue for window ww
                        if L == 0:
                            v.tensor_mul(ttsc[:], SA_s[:, ww, :], att_ap)
                            v.drain()
                            v.tensor_reduce(d_s[:, ww:ww + 1], ttsc[:],
                                            mybir.AxisListType.X, Alu.add)
                            v.tensor_scalar_mul(
                                hs_s[:, ww, 0:H], SA_s[:, ww, :],
                                g1d2_s[:, ww:ww + 1]).then_inc(s_fl[0], 1)
                        else:
                            v.tensor_scalar_mul(
                                h2_s[:, ww, :], SA_s[:, ww, :],
                                g2d_s[:, ww:ww + 1]).then_inc(s_fl[1], 1)

                for g in range(NG):
                    s0, nsg = p.group_slices[g]
                    for i in range(nsg):
                        sl = s0 + i
                        if sl >= RING:
                            v.wait_ge(s_mm[L],
                                      int(p.slice_group[sl - RING]) + 1)
                        v.tensor_scalar(S_s[:, sl % RING, :], io_s[:],
                                        cw_s[:, sl:sl + 1], None, Alu.is_equal
                                        ).then_inc(s_oh[L], 1)
                    if g >= 1:
                        flush(g - 1)
                flush(NG - 1)
                if L == 0:
                    # gate for layer 2: d1 = dot * g1d ; g2d = (sig*(1+e)-e)*dinv
                    v.drain()
                    v.tensor_mul(dt_s[:], d_s[:], g1d_s[:]).then_inc(s_fl[0], 1)
                    v.wait_ge(s_act2, 1)
                    v.tensor_scalar(gsc[:], gsc[:], 1.0 + EPS, -EPS,
                                    Alu.mult, Alu.add)
                    v.drain()
                    v.tensor_mul(g2d_s[:], gsc[:], dv_s[:])
                else:
                    # copy transposed h2 tiles into h2T
                    for ww in range(NT):
                        v.wait_ge(s_tr, ww + 1)
                        v.tensor_copy(h2T_s[:, ww * 128:(ww + 1) * 128],
                                      ptr[ww % 2][0:H, 0:128]).then_inc(s_h2c, 1)

            if L == 0:
                @block.scalar
                def _(a):
                    a.wait_ge(s_fl[0], NG + 1)
                    a.activation(gsc[:], dt_s[:], Act.Sigmoid).then_inc(s_act2, 1)

                @block.sync
                def _(s):
                    emit_hs_dma(s, 1, s_fl[0], base=NT)
            else:
                @block.scalar
                def _(a):
                    for j, (c0, cw) in enumerate(mlp_chunks):
                        a.wait_ge(s_om, j + 1)
                        a.activation(oT_s[:, c0:c0 + cw],
                                     pout[j % 2][0:F_OUT, 0:cw],
                                     Act.Identity, bias=b2_s[:]
                                     ).then_inc(s_oc, 1)

                @block.sync
                def _(s):
                    s.wait_ge(s_oc, len(mlp_chunks))
                    s.dma_start(out=out_e[:], in_=oT_s[:]).then_inc(s_out, 16)
                    s.wait_ge(s_out, 16)

    edge_layer(0)
    edge_layer(1)

    for t in reversed(ctx_tensors):
        t.__exit__(None, None, None)
    # Raw Bass skips codegen_inst_isa_subclasses; without it the ISA-opcode
    # instructions (load_library) reach walrus with empty payloads ->
    # "ISA wrong length".
    library_overlay.lower_extended_insts(nc)
    return nc


def _in_maps(p):
    maps = []
    for c in range(NC):
        maps.append({
            "xt": p.xt[c], "w1": p.w1, "b1bc": p.b1bc, "attbc": p.attbc,
            "w2": p.w2, "b2c": p.b2c, "ident": p.ident, "iota": p.iota,
            "dinv": p.dinv[c], "ri": p.ri[c], "colw": p.colw[c],
        })
    return maps


def kernel(x, edge_index, W1, b1, att, W2, b2, _trace=False):
    p = _preprocess(x, edge_index, W1, b1, att, W2, b2)
    nc = build_program(p)
    from concourse.bass_utils import run_bass_kernel_spmd
    if _trace:
        try:
            res = run_bass_kernel_spmd(nc, _in_maps(p), list(range(NC)),
                                       trace=True)
        except Exception as e:
            print(f"trace run failed ({e!r}); retrying without trace",
                  file=sys.stderr)
            _trace = False
    if not _trace:
        res = run_bass_kernel_spmd(nc, _in_maps(p), list(range(NC)))
    out = np.empty((N_NODES, F_OUT), dtype=np.float32)
    for c in range(NC):
        out[c * NPC:(c + 1) * NPC] = res.results[c]["outT"][:, :NPC].T
    if _trace:
        kernel.last_exec_time_ns = res.exec_time_ns
        kernel.last_results = res
    return out


# revision 18
# speedup vs baseline: 1.0942x; 1.0942x over previous
"""FAGCN (2-layer, eval mode) on 8 Trainium2 NeuronCores.

Self-contained kernel: takes FULL inputs, shards across 8 cores internally,
runs one SPMD Bass program, gathers the full output.

Math (per layer, derived from the reference):
  norm_e = dinv[row_e] * dinv[col_e]  factorizes, so with hs_i = dinv_i * h_i:
    x_l[c] = dinv_c * ( sum_{e->c} hs[row_e] + hs_c )        (self-loop folded in)
    gate_i = sigmoid(h_i . att_l)
    h_new  = (gate*(1+eps) - eps) * x_l                       (since x_h = -eps*x_l)

Device strategy per core (6250 owned nodes):
  - MLP: h0 = relu(x W1 + b1) on owned nodes (x transposed, bf16; b1 folded
    into the PE via a rank-1 ones x b1 matmul).
  - hs tables (bf16, 128-wide zero-padded rows = 256B gather elem) exchanged
    with two AllGathers per layer.
  - Propagate: dma_gather 256B rows per edge round-robined over all 4 SWDGE
    queues (each queue runs on its own Q7 core pair -> 4x descriptor-gen
    overlap) -> PE segment-sum with HOST-PRECOMPUTED bf16 one-hot matrices
    streamed from DRAM (no DVE is_equal work at all), 64-target half-window
    groups accumulating into disjoint PSUM partition halves.
  - Schedule runs in 4 sweeps [srcA w<25][srcB w<25][srcA w>=25][srcB w>=25]
    so windows 0..24 finish ~halfway through a layer and the next layer's
    A-table AllGather overlaps the remaining edge work.
  - Output: h2 tiles PE-transposed, out^T = W2^T @ h2^T + b2, host transposes.
"""

import math
import os
import sys

import numpy as np

for _p in ("/opt/trn_rl_repo",):
    if _p not in sys.path:
        sys.path.insert(0, _p)

import ml_dtypes

BF16 = ml_dtypes.bfloat16

# ----------------------------------------------------------------------------
# Problem constants (hardcoded per the harness contract)
# ----------------------------------------------------------------------------
N_NODES = 50000
N_EDGES = 800000
EPS = 0.1
NC = 8              # cores
F_IN = 256          # input features
H = 64              # hidden
F_OUT = 16
NPC = N_NODES // NC  # 6250 nodes per core
NT = (NPC + 127) // 128          # 49 windows / node tiles per core
NPADR = NT * 128                 # 6272 padded rows per core
WA = 25                          # windows in half A
ROWS_A = WA * 128                # 3200  (8*3200 = 25600 < 32767 for int16 idx)
ROWS_B = NPC - ROWS_A            # 3050  (8*3050 = 24400)
CH_SLICES = 8                    # slices per gather chunk (1024 edges = ring cap)
NBUF = 8                         # gather bounce buffers (multiple of NQ)
NQ = 4                           # SWDGE queues used round-robin
NSBUF = 4                        # one-hot S stream buffers
CHN = 512                        # MLP node-chunk (columns of x^T per load)
TW = 128                         # table row width (bf16), 256B rows


class _Plan:
    """Static (SPMD-identical) schedule + per-core input arrays."""
    pass


def _preprocess(x, edge_index, W1, b1, att, W2, b2):
    p = _Plan()
    row = np.asarray(edge_index[0], dtype=np.int64)
    col = np.asarray(edge_index[1], dtype=np.int64)

    deg = np.bincount(col, minlength=N_NODES).astype(np.float64) + 1.0
    dinv = (1.0 / np.sqrt(deg)).astype(np.float32)

    owner = col // NPC
    cl = (col % NPC).astype(np.int64)
    ww = cl // 128
    tloc = cl % 128
    so = (row % NPC).astype(np.int64)
    sown = row // NPC
    half = (so >= ROWS_A).astype(np.int64)
    gidx = np.where(half == 0, sown * ROWS_A + so, sown * ROWS_B + (so - ROWS_A))
    assert gidx.max() < 32768

    # group order: 4 sweeps x windows; one group == one flush pair
    GORDER = []           # gi -> (hh, ww)
    for sw in range(4):
        hh_s = sw % 2
        wlo, whi = (0, WA) if sw < 2 else (WA, NT)
        for wwi in range(wlo, whi):
            GORDER.append((hh_s, wwi))
    p.pairs = GORDER
    NGR = len(GORDER)
    p.NGR = NGR
    p.NPAIR = len(p.pairs)                   # == 2*NT
    gpos = {}
    for gi, gkey in enumerate(GORDER):
        gpos[gkey] = gi
    sweep_of_group = np.empty(NGR, dtype=np.int64)
    for gi, (hh_s, wwi) in enumerate(GORDER):
        sweep_of_group[gi] = (0 if wwi < WA else 2) + hh_s
    # flush position where window w is fully accumulated (1-based s_fl count)
    p.fl_done = [0] * NT
    for pi, (hh_s, wwi) in enumerate(p.pairs):
        if hh_s == 1:
            p.fl_done[wwi] = pi + 1

    # per-edge group id
    gid_lut = np.empty((2, NT), dtype=np.int64)
    for (hh_s, wwi), gi in gpos.items():
        gid_lut[hh_s, wwi] = gi
    egid = gid_lut[half, ww]

    # sort edges by (owner, group)
    order = np.lexsort((egid, owner))
    so_own, so_gid, so_gidx2, so_tloc = (
        owner[order], egid[order], gidx[order], tloc[order])

    # group counts per (core, group); slices = max over cores
    key = so_own * NGR + so_gid
    cnt = np.bincount(key, minlength=NC * NGR).reshape(NC, NGR)
    nsl = np.maximum(1, (cnt.max(axis=0) + 127) // 128)     # [NGR]
    p.group_slices = []
    acc = 0
    for gi in range(NGR):
        p.group_slices.append((acc, int(nsl[gi])))
        acc += int(nsl[gi])
    p.TOT_SLICES = acc
    p.slice_group = np.empty(acc, dtype=np.int64)
    for gi, (s0, ns) in enumerate(p.group_slices):
        p.slice_group[s0:s0 + ns] = gi

    # chunks: contiguous slices, never crossing a sweep boundary
    p.chunks = []          # (sweep, start_slice, n_slices)
    gi = 0
    for sw in range(4):
        lo = p.group_slices[gi][0]
        while gi < NGR and sweep_of_group[gi] == sw:
            gi += 1
        hi = p.group_slices[gi][0] if gi < NGR else acc
        s = lo
        while s < hi:
            ns = min(CH_SLICES, hi - s)
            p.chunks.append((sw, s, ns))
            s += ns
    p.slice_chunk = np.empty(acc, dtype=np.int64)
    p.chunk_start = np.empty(len(p.chunks), dtype=np.int64)
    for ci, (sw, s0, ns) in enumerate(p.chunks):
        p.slice_chunk[s0:s0 + ns] = ci
        p.chunk_start[ci] = s0
    p.SPLIT = next(s0 for (sw, s0, ns) in p.chunks if s0 >= acc // 2)

    # per-core edge index array (wrapped for the gather) + one-hot stream
    p.ri = np.zeros((NC, 128, p.TOT_SLICES * 8), dtype=np.int16)
    p.sh = np.zeros((NC, 128, p.TOT_SLICES, 128), dtype=BF16)
    seg_starts = np.zeros(NC * NGR + 1, dtype=np.int64)
    np.cumsum(cnt.reshape(-1), out=seg_starts[1:])
    slice_s0 = np.array([s0 for s0, _ in p.group_slices], dtype=np.int64)
    for c in range(NC):
        ri_flat = np.zeros(p.TOT_SLICES * 128, dtype=np.int16)
        a0 = seg_starts[c * NGR]
        a1 = seg_starts[(c + 1) * NGR]
        # linear position of each edge within the padded slice layout
        seg_off = np.arange(a0, a1) - seg_starts[c * NGR + so_gid[a0:a1]]
        lin = slice_s0[so_gid[a0:a1]] * 128 + seg_off
        ri_flat[lin] = so_gidx2[a0:a1]
        pp = lin % 128
        sl = lin // 128
        p.sh[c, pp, sl, so_tloc[a0:a1]] = 1.0
        # wrap ri per chunk: idx stream layout [16 partitions, free] x8 groups
        for ci, (sw, s0, ns) in enumerate(p.chunks):
            blk = ri_flat[s0 * 128:(s0 + ns) * 128]
            wrapped = blk.reshape(ns * 8, 16).T          # [16, ns*8]
            p.ri[c, :, s0 * 8:(s0 + ns) * 8] = np.tile(wrapped, (8, 1))
    p.sh = p.sh.reshape(NC, 128, p.TOT_SLICES * 128)

    # per-core dense inputs
    p.xt = np.zeros((NC, 2, 128, NPADR), dtype=BF16)
    p.dinv = np.zeros((NC, 128, NT), dtype=np.float32)
    x = np.asarray(x, dtype=np.float32)
    for c in range(NC):
        xs = x[c * NPC:(c + 1) * NPC]                    # [6250, 256]
        xts = np.zeros((F_IN, NPADR), dtype=np.float32)
        xts[:, :NPC] = xs.T
        p.xt[c] = xts.reshape(2, 128, NPADR).astype(BF16)
        dv = np.zeros(NPADR, dtype=np.float32)
        dv[:NPC] = dinv[c * NPC:(c + 1) * NPC]
        p.dinv[c] = dv.reshape(NT, 128).T

    # shared weights / constants
    W1 = np.asarray(W1, dtype=np.float32)
    p.w1 = np.concatenate([W1[:128, :], W1[128:, :]], axis=1).astype(BF16)
    p.b1bc = np.tile(np.asarray(b1, dtype=np.float32)[None, :], (128, 1))
    p.has_b1 = bool(np.any(np.asarray(b1)))
    att = np.asarray(att, dtype=np.float32).reshape(2, H)
    p.attbc = np.tile(att.reshape(1, 2 * H), (128, 1))          # [128, 2H]
    p.w2 = np.asarray(W2, dtype=np.float32).astype(BF16)        # [64, 16]
    p.b2c = np.asarray(b2, dtype=np.float32).reshape(F_OUT, 1)
    p.ident = np.eye(128, dtype=np.float32)
    return p


# ----------------------------------------------------------------------------
# Bass program
# ----------------------------------------------------------------------------

def build_program(p):
    from concourse import bass, mybir
    from concourse import library_config, library_overlay

    f32 = mybir.dt.float32
    bf16 = mybir.dt.bfloat16
    i16 = mybir.dt.int16
    Alu = mybir.AluOpType
    Act = mybir.ActivationFunctionType

    nc = bass.Bass(trn_type="TRN2", num_devices=NC, num_swdge_queues=NQ)
    cores = list(range(NC))

    # ---- external I/O
    xt_e = nc.declare_dram_parameter("xt", [2, 128, NPADR], bf16, isOutput=False)
    w1_e = nc.declare_dram_parameter("w1", [128, 2 * H], bf16, isOutput=False)
    b1_e = nc.declare_dram_parameter("b1bc", [128, H], f32, isOutput=False)
    att_e = nc.declare_dram_parameter("attbc", [128, 2 * H], f32, isOutput=False)
    w2_e = nc.declare_dram_parameter("w2", [H, F_OUT], bf16, isOutput=False)
    b2_e = nc.declare_dram_parameter("b2c", [F_OUT, 1], f32, isOutput=False)
    id_e = nc.declare_dram_parameter("ident", [128, 128], f32, isOutput=False)
    dv_e = nc.declare_dram_parameter("dinv", [128, NT], f32, isOutput=False)
    ri_e = nc.declare_dram_parameter("ri", [128, p.TOT_SLICES * 8], i16,
                                     isOutput=False)
    sh0_e = nc.declare_dram_parameter("sh0", [128, p.SPLIT * 128], bf16,
                                      isOutput=False)
    sh1_e = nc.declare_dram_parameter("sh1", [128, (p.TOT_SLICES - p.SPLIT) * 128],
                                      bf16, isOutput=False)

    def sh_ap(s0, ns):
        if s0 < p.SPLIT:
            return sh0_e[:, s0 * 128:(s0 + ns) * 128]
        return sh1_e[:, (s0 - p.SPLIT) * 128:(s0 - p.SPLIT + ns) * 128]
    out_e = nc.declare_dram_parameter("outT", [F_OUT, NPADR], f32, isOutput=True)

    # ---- internal DRAM (bf16 tables, 256B rows)
    hsA = [nc.dram_tensor(f"hs{L}A", [ROWS_A, TW], bf16) for L in range(2)]
    hsB = [nc.dram_tensor(f"hs{L}B", [ROWS_B, TW], bf16) for L in range(2)]
    tA = [nc.dram_tensor(f"t{L}A", [NC * ROWS_A, TW], bf16, addr_space="Shared")
          for L in range(2)]
    tB = [nc.dram_tensor(f"t{L}B", [NC * ROWS_B, TW], bf16, addr_space="Shared")
          for L in range(2)]

    ctx_tensors = []

    def sb(name, shape, dt=f32):
        t = nc.sbuf_tensor(name, shape, dt)
        ctx_tensors.append(t)
        return t.__enter__()

    def ps(name):
        t = nc.psum_tensor(name, [128, 512], f32)
        ctx_tensors.append(t)
        return t.__enter__()

    def sem(name):
        s = nc.semaphore(name)
        ctx_tensors.append(s)
        return s.__enter__()

    # ---- SBUF
    xc = [sb(f"xc{i}", [128, 2, CHN], bf16) for i in range(2)]
    w1_s = sb("w1s", [128, 2 * H], bf16)
    b1_s = sb("b1s", [128, H])
    att_s = sb("atts", [128, 2 * H])
    w2_s = sb("w2s", [H, F_OUT], bf16)
    b2_s = sb("b2s", [F_OUT, 1])
    id_s = sb("ids", [128, 128])
    dv_s = sb("dvs", [128, NT])
    ri_s = sb("ris", [128, p.TOT_SLICES * 8], i16)
    gb = [sb(f"gb{i}", [128, CH_SLICES, TW], bf16) for i in range(NBUF)]
    sc = [sb(f"sc{i}", [128, CH_SLICES, 128], bf16) for i in range(NSBUF)]
    SA_s = sb("SAs", [128, NT, H])
    hs_s = sb("hss", [128, NT, TW], bf16)      # cols H..TW stay zero
    h2_s = sb("h2s", [128, NT, H])
    h0sc = sb("h0sc", [128, H])
    ttsc = sb("ttsc", [128, H])
    d_s = sb("ds", [128, NT])
    dt_s = sb("dts", [128, NT])
    g1d_s = sb("g1ds", [128, NT])
    g1d2_s = sb("g1d2s", [128, NT])
    g2d_s = sb("g2ds", [128, NT])
    gsc = sb("gsc", [128, NT])
    h2T_s = sb("h2Ts", [H, NPADR], bf16)
    oT_s = sb("oTs", [F_OUT, NPADR])

    pseg = [ps(f"pg{i}") for i in range(4)]
    ptr = [ps(f"pt{i}") for i in range(2)]
    pout = [ps(f"po{i}") for i in range(2)]

    # ---- semaphores
    s_ld = sem("s_ld")
    s_xtp = [sem("s_xt0"), sem("s_xt1")]
    s_pe1 = sem("s_pe1")
    s_hs = sem("s_hs")
    s_act1 = sem("s_act1")
    s_cc = sem("s_cc")
    s_hsdA = [sem(f"s_hsdA{L}") for L in range(2)]
    s_hsdB = [sem(f"s_hsdB{L}") for L in range(2)]
    s_g = [[sem(f"s_g{L}_{i}") for i in range(NBUF)] for L in range(2)]
    s_sh = [[sem(f"s_sh{L}_{i}") for i in range(NSBUF)] for L in range(2)]
    s_mm = [sem(f"s_mm{L}") for L in range(2)]
    s_fl = [sem(f"s_fl{L}") for L in range(2)]
    s_act2 = sem("s_act2")
    s_tr = sem("s_tr")
    s_h2c = sem("s_h2c")
    s_om = sem("s_om")
    s_oc = sem("s_oc")
    s_out = sem("s_out")

    NPAIR = p.NPAIR          # flushes per layer (2*NT)

    # hs DMA out, one 2D DMA per window. A-half windows [0, WA) increment
    # s_hsdA[L] by 16 each; the rest s_hsdB[L].
    BP = ROWS_B - (NT - 1 - WA) * 128      # rows in the partial last window

    def emit_hs_window(s, L, ww, wait=True):
        if wait:
            s.wait_ge(s_fl[0] if L == 1 else s_hs,
                      p.fl_done[ww] if L == 1 else ww + 1)
        if ww < WA:
            dst = hsA[L][ww * 128:(ww + 1) * 128, :]
            src = hs_s[:, ww, :]
        elif ww < NT - 1:
            r0 = (ww - WA) * 128
            dst = hsB[L][r0:r0 + 128, :]
            src = hs_s[:, ww, :]
        else:
            r0 = (ww - WA) * 128
            dst = hsB[L][r0:r0 + BP, :]
            src = hs_s[0:BP, ww, :]
        s.dma_start(out=dst, in_=src).then_inc(
            s_hsdA[L] if ww < WA else s_hsdB[L], 16)

    def emit_hs_dma(s, L, gate_sem, thresholds):
        for ww in range(NT):
            s.wait_ge(gate_sem, thresholds[ww])
            emit_hs_window(s, L, ww, wait=False)

    # MLP chunk bookkeeping
    mlp_chunks = []
    c0 = 0
    while c0 < NPADR:
        cw = min(CHN, NPADR - c0)
        mlp_chunks.append((c0, cw))
        c0 += cw
    tiles_before_chunk = [0]
    for (c0, cw) in mlp_chunks:
        tiles_before_chunk.append(tiles_before_chunk[-1] + cw // 128)

    # ======================================================================
    # Block 0: constant loads + hs pad-zeroing
    # ======================================================================
    with nc.Block() as block:
        @block.gpsimd
        def _(g):
            g.load_library(library_config.mlp)

        @block.vector
        def _(v):
            # zero hs_s once (pad cols H..TW must stay zero); contiguous AP
            v.memset(hs_s[:], 0.0)

        @block.sync
        def _(s):
            n = 0
            for dst, src in [
                (w1_s[:], w1_e[:]), (b1_s[:], b1_e[:]),
                (att_s[:], att_e[:]), (w2_s[:], w2_e[:]), (b2_s[:], b2_e[:]),
                (id_s[:], id_e[:]), (dv_s[:], dv_e[:]), (ri_s[:], ri_e[:]),
            ]:
                s.dma_start(out=dst, in_=src).then_inc(s_ld, 16)
                n += 16
            s.wait_ge(s_ld, n)

    # ======================================================================
    # Block 1: MLP -> hs0, first S-chunk preloads, hs0 DMA out, AG for layer 0
    # ======================================================================
    with nc.Block() as block:
        @block.sync
        def _(s):
            # preload the first S chunks for layer 0 before anything else
            for ci in range(min(NSBUF, len(p.chunks))):
                sw, s0, ns = p.chunks[ci]
                s.dma_start(out=sc[ci % NSBUF][:, 0:ns, :],
                            in_=sh_ap(s0, ns)
                            ).then_inc(s_sh[0][ci % NSBUF], 16)
            for j, (c0, cw) in enumerate(mlp_chunks):
                if j >= 2:
                    s.wait_ge(s_pe1, tiles_before_chunk[j - 1])
                for hh in range(2):
                    s.dma_start(out=xc[j % 2][:, hh, 0:cw],
                                in_=xt_e[hh, :, c0:c0 + cw]
                                ).then_inc(s_xtp[j % 2], 16)
            # hs0 windows out (after DVE produced them)
            emit_hs_dma(s, 0, s_hs, [ww + 1 for ww in range(NT)])

        @block.tensor
        def _(t):
            for ti in range(NT):
                j = None
                for jj, (c0, cw) in enumerate(mlp_chunks):
                    if ti * 128 >= c0 and ti * 128 < c0 + cw:
                        j = jj
                        u = (ti * 128 - c0) // 128
                        break
                t.wait_ge(s_xtp[j % 2], 32 * (j // 2 + 1))
                if ti >= 4:
                    t.wait_ge(s_hs, ti - 3)
                t.matmul(pseg[ti % 4][:, 0:H],
                         xc[j % 2][:, 0, u * 128:(u + 1) * 128],
                         w1_s[:, 0:H], start=True, stop=False)
                t.matmul(pseg[ti % 4][:, 0:H],
                         xc[j % 2][:, 1, u * 128:(u + 1) * 128],
                         w1_s[:, H:2 * H], start=False, stop=True
                         ).then_inc(s_pe1, 1)

        @block.vector
        def _(v):
            for ti in range(NT):
                v.wait_ge(s_pe1, ti + 1)
                if ti >= 1:
                    v.wait_ge(s_hs, ti)       # self-sync scratch reuse
                if p.has_b1:
                    v.tensor_add(h0sc[:], pseg[ti % 4][:, 0:H], b1_s[:])
                    v.drain()
                    v.tensor_scalar_max(h0sc[:], h0sc[:], 0.0)
                else:
                    v.tensor_scalar_max(h0sc[:], pseg[ti % 4][:, 0:H], 0.0)
                v.drain()
                v.tensor_mul(ttsc[:], h0sc[:], att_s[:, 0:H])
                v.drain()
                v.tensor_reduce(d_s[:, ti:ti + 1], ttsc[:],
                                mybir.AxisListType.X, Alu.add)
                v.tensor_scalar_mul(hs_s[:, ti, 0:H], h0sc[:],
                                    dv_s[:, ti:ti + 1]).then_inc(s_hs, 1)
            v.wait_ge(s_act1, 1)
            v.tensor_scalar(gsc[:], gsc[:], 1.0 + EPS, -EPS, Alu.mult, Alu.add)
            v.drain()
            v.tensor_mul(g1d_s[:], gsc[:], dv_s[:])
            v.drain()
            v.tensor_mul(g1d2_s[:], g1d_s[:], dv_s[:])

        @block.scalar
        def _(a):
            a.wait_ge(s_hs, NT)
            a.activation(gsc[:], d_s[:], Act.Sigmoid).then_inc(s_act1, 1)

        @block.gpsimd
        def _(g):
            g.wait_ge(s_hsdA[0], 16 * WA)
            g.collective_compute(
                "AllGather", Alu.bypass, replica_groups=[cores],
                ins=[hsA[0][:]], outs=[tA[0][:]]).then_inc(s_cc, 1)
            g.wait_ge(s_hsdB[0], 16 * (NT - WA))
            g.collective_compute(
                "AllGather", Alu.bypass, replica_groups=[cores],
                ins=[hsB[0][:]], outs=[tB[0][:]]).then_inc(s_cc, 1)

    # ======================================================================
    # Blocks 2/3: edge layers
    # ======================================================================
    def edge_layer(L):
        att_ap = att_s[:, H:2 * H]      # att for NEXT gate (layer L+1)
        with nc.Block() as block:
            @block.gpsimd
            def _(g):
                seen_b = False
                size_regs = {}
                for ci, (sw, s0, ns) in enumerate(p.chunks):
                    if ns * 128 not in size_regs:
                        size_regs[ns * 128] = g.to_reg(ns * 128)
                    if ci == 0:
                        g.wait_ge(s_cc, 2 * L + 1)
                    if sw % 2 == 1 and not seen_b:
                        g.wait_ge(s_cc, 2 * L + 2)
                        seen_b = True
                    if ci >= NBUF:
                        last_sl = int(p.chunk_start[ci - NBUF]
                                      + p.chunks[ci - NBUF][2] - 1)
                        g.wait_ge(s_mm[L], int(p.slice_group[last_sl]) + 1)
                    elif L == 1:
                        # buffer last used by layer-0's tail chunks
                        prev_ci = len(p.chunks) - NBUF + ci
                        last_sl = int(p.chunk_start[prev_ci]
                                      + p.chunks[prev_ci][2] - 1)
                        g.wait_ge(s_mm[0], int(p.slice_group[last_sl]) + 1)
                    tab = (tA, tB)[sw % 2][L]
                    g.dma_gather(
                        gb[ci % NBUF][:, 0:ns, :], tab[:],
                        ri_s[:, s0 * 8:(s0 + ns) * 8],
                        ns * 128, size_regs[ns * 128], TW,
                        queue_num=ci % NQ,
                        ).then_inc(s_g[L][ci % NBUF], 16)
                # issue next layer's AllGathers (after hs(L+1) DMA'd out)
                if L == 0:
                    g.wait_ge(s_hsdA[1], 16 * WA)
                    g.collective_compute(
                        "AllGather", Alu.bypass, replica_groups=[cores],
                        ins=[hsA[1][:]], outs=[tA[1][:]]).then_inc(s_cc, 1)
                    g.wait_ge(s_hsdB[1], 16 * (NT - WA))
                    g.collective_compute(
                        "AllGather", Alu.bypass, replica_groups=[cores],
                        ins=[hsB[1][:]], outs=[tB[1][:]]).then_inc(s_cc, 1)

            @block.scalar
            def _(a):
                if L == 0:
                    a.wait_ge(s_fl[0], NPAIR + 1)
                    a.activation(gsc[:], dt_s[:], Act.Sigmoid
                                 ).then_inc(s_act2, 1)
                else:
                    for j, (c0, cw) in enumerate(mlp_chunks):
                        a.wait_ge(s_om, j + 1)
                        a.activation(oT_s[:, c0:c0 + cw],
                                     pout[j % 2][0:F_OUT, 0:cw],
                                     Act.Identity, bias=b2_s[:]
                                     ).then_inc(s_oc, 1)

            @block.tensor
            def _(t):
                last_g = [[0] for _ in range(NBUF)]
                last_s = [[0] for _ in range(NSBUF)]

                def wge(eng, se, th, tracker):
                    if th > tracker[0]:
                        eng.wait_ge(se, th)
                        tracker[0] = th
                for pi, (hh, ww) in enumerate(p.pairs):
                    if pi >= 4:
                        t.wait_ge(s_fl[L], pi - 3)
                    elif L == 1:
                        # pseg bank pi%4 last used by a layer-0 pair
                        q = max(q for q in range(NPAIR) if q % 4 == pi % 4)
                        t.wait_ge(s_fl[0], q + 1)
                    else:
                        # pseg bank pi%4 last used by an MLP tile (DVE reads
                        # it at s_hs == tile+1)
                        q = max(q for q in range(NT) if q % 4 == pi % 4)
                        t.wait_ge(s_hs, q + 1)
                    s0, nsg = p.group_slices[pi]
                    for i in range(nsg):
                        sl = s0 + i
                        ci = int(p.slice_chunk[sl])
                        wge(t, s_g[L][ci % NBUF], 16 * (ci // NBUF + 1),
                            last_g[ci % NBUF])
                        wge(t, s_sh[L][ci % NSBUF], 16 * (ci // NSBUF + 1),
                            last_s[ci % NSBUF])
                        j = sl - int(p.chunk_start[ci])
                        mm = t.matmul(
                            pseg[pi % 4][:, 0:H],
                            sc[ci % NSBUF][:, j, :],
                            gb[ci % NBUF][:, j, 0:H],
                            start=(i == 0), stop=(i == nsg - 1))
                        if i == nsg - 1:
                            mm.then_inc(s_mm[L], 1)
                # layer-2 extras: transposes + final matmul
                if L == 1:
                    for ww in range(NT):
                        t.wait_ge(s_fl[1], p.fl_done[ww])
                        if ww >= 2:
                            t.wait_ge(s_h2c, ww - 1)
                        t.transpose(ptr[ww % 2][0:H, 0:128], h2_s[:, ww, :],
                                    id_s[:]).then_inc(s_tr, 1)
                    for j, (c0, cw) in enumerate(mlp_chunks):
                        t.wait_ge(s_h2c, min(NT, tiles_before_chunk[j + 1]))
                        if j >= 2:
                            t.wait_ge(s_oc, j - 1)
                        t.matmul(pout[j % 2][0:F_OUT, 0:cw], w2_s[:],
                                 h2T_s[:, c0:c0 + cw], start=True, stop=True
                                 ).then_inc(s_om, 1)

            @block.vector
            def _(v):
                for pi, (hh, ww) in enumerate(p.pairs):
                    v.wait_ge(s_mm[L], pi + 1)
                    if hh == 0:
                        v.tensor_add(SA_s[:, ww, :], pseg[pi % 4][:, 0:H],
                                     hs_s[:, ww, 0:H]).then_inc(s_fl[L], 1)
                    else:
                        v.tensor_add(SA_s[:, ww, :], SA_s[:, ww, :],
                                     pseg[pi % 4][:, 0:H])
                        v.drain()
                        if L == 0:
                            v.tensor_mul(ttsc[:], SA_s[:, ww, :], att_ap)
                            v.drain()
                            v.tensor_reduce(d_s[:, ww:ww + 1], ttsc[:],
                                            mybir.AxisListType.X, Alu.add)
                            v.tensor_scalar_mul(
                                hs_s[:, ww, 0:H], SA_s[:, ww, :],
                                g1d2_s[:, ww:ww + 1]).then_inc(s_fl[0], 1)
                        else:
                            v.tensor_scalar_mul(
                                h2_s[:, ww, :], SA_s[:, ww, :],
                                g2d_s[:, ww:ww + 1]).then_inc(s_fl[1], 1)
                if L == 0:
                    # gate for layer 2: d1 = dot * g1d ; g2d = (sig*(1+e)-e)*dinv
                    v.drain()
                    v.tensor_mul(dt_s[:], d_s[:], g1d_s[:]).then_inc(s_fl[0], 1)
                    v.wait_ge(s_act2, 1)
                    v.tensor_scalar(gsc[:], gsc[:], 1.0 + EPS, -EPS,
                                    Alu.mult, Alu.add)
                    v.drain()
                    v.tensor_mul(g2d_s[:], gsc[:], dv_s[:])
                else:
                    # copy transposed h2 tiles into h2T
                    for ww in range(NT):
                        v.wait_ge(s_tr, ww + 1)
                        v.tensor_copy(h2T_s[:, ww * 128:(ww + 1) * 128],
                                      ptr[ww % 2][0:H, 0:128]).then_inc(s_h2c, 1)

            def s_load_events():
                ev = []
                for ci, (sw, s0, ns) in enumerate(p.chunks):
                    if L == 0 and ci < NSBUF:
                        continue
                    if ci >= NSBUF:
                        last_sl = int(p.chunk_start[ci - NSBUF]
                                      + p.chunks[ci - NSBUF][2] - 1)
                        w = (s_mm[L], int(p.slice_group[last_sl]) + 1)
                    else:
                        # L == 1: buffer last used by layer-0's tail chunks
                        prev_ci = len(p.chunks) - NSBUF + ci
                        last_sl = int(p.chunk_start[prev_ci]
                                      + p.chunks[prev_ci][2] - 1)
                        w = (s_mm[0], int(p.slice_group[last_sl]) + 1)
                    ev.append((w[1] if (ci >= NSBUF or L == 0) else -1, 0,
                               ('sl', ci, w)))
                return ev

            if L == 0:
                @block.sync
                def _(s):
                    ev = s_load_events()
                    for ww in range(NT):
                        ev.append((p.fl_done[ww], 1, ('hs', ww, None)))
                    ev.sort(key=lambda e: (e[0], e[1]))
                    for _, _, (kind, idx, w) in ev:
                        if kind == 'sl':
                            sw, s0, ns = p.chunks[idx]
                            s.wait_ge(w[0], w[1])
                            s.dma_start(out=sc[idx % NSBUF][:, 0:ns, :],
                                        in_=sh_ap(s0, ns)
                                        ).then_inc(s_sh[0][idx % NSBUF], 16)
                        else:
                            emit_hs_window(s, 1, idx)
            else:
                @block.sync
                def _(s):
                    for _, _, (kind, idx, w) in sorted(
                            s_load_events(), key=lambda e: (e[0], e[1])):
                        sw, s0, ns = p.chunks[idx]
                        s.wait_ge(w[0], w[1])
                        s.dma_start(out=sc[idx % NSBUF][:, 0:ns, :],
                                    in_=sh_ap(s0, ns)
                                    ).then_inc(s_sh[1][idx % NSBUF], 16)
                    s.wait_ge(s_oc, len(mlp_chunks))
                    s.dma_start(out=out_e[:], in_=oT_s[:]).then_inc(s_out, 16)
                    s.wait_ge(s_out, 16)

    edge_layer(0)
    edge_layer(1)

    for t in reversed(ctx_tensors):
        t.__exit__(None, None, None)
    # Raw Bass skips codegen_inst_isa_subclasses; without it the ISA-opcode
    # instructions (load_library) reach walrus with empty payloads ->
    # "ISA wrong length".
    library_overlay.lower_extended_insts(nc)
    return nc


def _in_maps(p):
    maps = []
    for c in range(NC):
        maps.append({
            "xt": p.xt[c], "w1": p.w1, "b1bc": p.b1bc,
            "attbc": p.attbc, "w2": p.w2, "b2c": p.b2c, "ident": p.ident,
            "dinv": p.dinv[c], "ri": p.ri[c],
            "sh0": p.sh[c][:, :p.SPLIT * 128],
            "sh1": p.sh[c][:, p.SPLIT * 128:],
        })
    return maps


def kernel(x, edge_index, W1, b1, att, W2, b2, _trace=False):
    p = _preprocess(x, edge_index, W1, b1, att, W2, b2)
    nc = build_program(p)
    from concourse.bass_utils import run_bass_kernel_spmd
    if _trace:
        try:
            res = run_bass_kernel_spmd(nc, _in_maps(p), list(range(NC)),
                                       trace=True)
        except Exception as e:
            print(f"trace run failed ({e!r}); retrying without trace",
                  file=sys.stderr)
            _trace = False
    if not _trace:
        res = run_bass_kernel_spmd(nc, _in_maps(p), list(range(NC)))
    out = np.empty((N_NODES, F_OUT), dtype=np.float32)
    for c in range(NC):
        out[c * NPC:(c + 1) * NPC] = res.results[c]["outT"][:, :NPC].T
    if _trace:
        kernel.last_exec_time_ns = res.exec_time_ns
        kernel.last_results = res
    return out
